# revision 7
# baseline (speedup 1.0000x reference)
"""Sliding-window (radius-8, K=17) single-head attention along W.

Full problem: feature/position [2, 128, 64, 256] f32; 1x1 convs Wq/Wk (+bias)
produce q/k; scores over a 17-wide window along W; softmax (zero-padded
windows contribute exp(0)=1 to the denominator); output is the attn-weighted
sum of windows of x = feature + position.

Sharding: data-parallel over (B, H) — the 128 (b, h) rows are independent;
each of the 8 cores gets 16 rows. Per row (x_row = [C=128, W=256]):
  q = (Wq/sqrt(C)) x + bq/sqrt(C);  k = Wk x + bk          (PE matmuls + bias)
  S[w, w'] = q^T k  (full 256x256, 2 matmuls; band extracted via mask)
  att = exp(S + bandmask); denom = rowsum(att) + n_oob; att *= 1/denom
  out = x @ att^T   (PE transposes of att and x, then 2 accumulating matmuls)

The softmax skips max-subtraction: scores are O(10) for this distribution, so
exp() is well within fp32 range, and out-of-band entries are masked to -1e9.
Zero-padded (out-of-range) window positions are accounted by adding their
exact count (n_oob, exp(0)=1 each) to the denominator.
"""

import numpy as np
from contextlib import ExitStack

import concourse.bass as bass
import concourse.bacc as bacc
import concourse.mybir as mybir
import concourse.tile as tile
from concourse.bass_utils import run_bass_kernel_spmd

B, C, H, W = 2, 128, 64, 256
R = 8
NCORES = 8
ROWS = B * H // NCORES        # 16 (b, h) rows per core
CORES_PER_B = NCORES // B     # 4
F32 = mybir.dt.float32
F32R = mybir.dt.float32r
EXP = mybir.ActivationFunctionType.Exp
NEG = -1e9
RL = 4                        # rows per input DMA
RS = 2                        # rows per output DMA


def _r(ap):
    return ap.bitcast(F32R)


def build_nc():
    nc = bacc.Bacc(trn_type="TRN2")
    f_ext = nc.dram_tensor("feature", [C, ROWS, W], F32, kind="ExternalInput")
    p_ext = nc.dram_tensor("position", [C, ROWS, W], F32, kind="ExternalInput")
    wq_ext = nc.dram_tensor("wqt", [C, C], F32R, kind="ExternalInput")
    wk_ext = nc.dram_tensor("wkt", [C, C], F32R, kind="ExternalInput")
    id_ext = nc.dram_tensor("ident", [C, C], F32R, kind="ExternalInput")
    bq_ext = nc.dram_tensor("bqv", [C, 1], F32, kind="ExternalInput")
    bk_ext = nc.dram_tensor("bkv", [C, 1], F32, kind="ExternalInput")
    mask_ext = nc.dram_tensor("bandmask", [C, 2 * W], F32, kind="ExternalInput")
    oob_ext = nc.dram_tensor("oob", [C, 2], F32, kind="ExternalInput")
    out_ext = nc.dram_tensor("out", [C, ROWS, W], F32, kind="ExternalOutput")

    with tile.TileContext(nc) as tc, ExitStack() as ctx:
        const = ctx.enter_context(tc.tile_pool(name="const", bufs=1))
        wq_t = const.tile([C, C], F32R)
        nc.sync.dma_start(wq_t[:], wq_ext[:])
        wk_t = const.tile([C, C], F32R)
        nc.sync.dma_start(wk_t[:], wk_ext[:])
        bq_t = const.tile([C, 1], F32)
        nc.sync.dma_start(bq_t[:], bq_ext[:])
        bk_t = const.tile([C, 1], F32)
        nc.sync.dma_start(bk_t[:], bk_ext[:])
        mask_t = const.tile([C, 2 * W], F32)
        nc.sync.dma_start(mask_t[:], mask_ext[:])
        oob_t = const.tile([C, 2], F32)
        nc.sync.dma_start(oob_t[:], oob_ext[:])
        ident = const.tile([C, C], F32R)
        nc.sync.dma_start(ident[:], id_ext[:])

        inp = ctx.enter_context(tc.tile_pool(name="inp", bufs=2))
        xp = ctx.enter_context(tc.tile_pool(name="x", bufs=3))
        qkp = ctx.enter_context(tc.tile_pool(name="qk", bufs=3))
        attp = ctx.enter_context(tc.tile_pool(name="att", bufs=3))
        smallp = ctx.enter_context(tc.tile_pool(name="small", bufs=4))
        sbT = ctx.enter_context(tc.tile_pool(name="sbT", bufs=3))
        ps = ctx.enter_context(tc.tile_pool(name="ps", bufs=2, space="PSUM"))

        ft = pt = o_ps = None
        for r in range(ROWS):
            if r % RL == 0:
                ft = inp.tile([C, RL, W], F32, tag="ft")
                nc.sync.dma_start(ft[:], f_ext[:, r : r + RL, :])
                pt = inp.tile([C, RL, W], F32, tag="pt")
                nc.sync.dma_start(pt[:], p_ext[:, r : r + RL, :])
            j = r % RL

            xt = xp.tile([C, W], F32R)
            nc.vector.tensor_add(_r(xt[:]), ft[:, j, :], pt[:, j, :])

            qk_ps = ps.tile([C, 2 * W], F32, tag="qk")
            nc.tensor.matmul(
                qk_ps[:, 0:W], _r(wq_t[:]), _r(xt[:]), start=True, stop=True
            )
            nc.tensor.matmul(
                qk_ps[:, W : 2 * W], _r(wk_t[:]), _r(xt[:]), start=True, stop=True
            )

            qk_sb = qkp.tile([C, 2 * W], F32R)
            nc.scalar.add(_r(qk_sb[:, 0:W]), qk_ps[:, 0:W], bq_t[:])
            nc.vector.tensor_scalar_add(_r(qk_sb[:, W : 2 * W]), qk_ps[:, W : 2 * W], bk_t[:])

            s_ps = ps.tile([C, 2 * W], F32, tag="s")
            nc.tensor.matmul(
                s_ps[:, 0:W],
                _r(qk_sb[:, 0:128]),
                _r(qk_sb[:, W : 2 * W]),
                start=True,
                stop=True,
            )
            nc.tensor.matmul(
                s_ps[:, W : 2 * W],
                _r(qk_sb[:, 128:256]),
                _r(qk_sb[:, W : 2 * W]),
                start=True,
                stop=True,
            )

            att = attp.tile([C, 2 * W], F32R)
            nc.vector.tensor_add(att[:], s_ps[:], mask_t[:])

            den = smallp.tile([C, 2], F32, tag="den")
            nc.scalar.activation(att[:, 0:W], att[:, 0:W], EXP, accum_out=den[:, 0:1])
            nc.scalar.activation(
                att[:, W : 2 * W], att[:, W : 2 * W], EXP, accum_out=den[:, 1:2]
            )

            rden = smallp.tile([C, 2], F32, tag="rden")
            nc.vector.tensor_add(rden[:], den[:], oob_t[:])
            nc.vector.reciprocal(rden[:], rden[:])

            nc.vector.tensor_scalar_mul(_r(att[:, 0:W]), att[:, 0:W], rden[:, 0:1])
            nc.vector.tensor_scalar_mul(
                _r(att[:, W : 2 * W]), att[:, W : 2 * W], rden[:, 1:2]
            )

            # attT = [C0 | C1]: C0 rows are keys w' 0:128, C1 rows keys 128:256;
            # columns are queries w 0:256.
            at_ps = ps.tile([C, 2 * W], F32, tag="tstage")
            nc.tensor.transpose(_r(at_ps[:, 0:128]), _r(att[:, 0:128]), _r(ident[:]))
            nc.tensor.transpose(_r(at_ps[:, 128:256]), _r(att[:, 256:384]), _r(ident[:]))
            nc.tensor.transpose(_r(at_ps[:, 256:384]), _r(att[:, 128:256]), _r(ident[:]))
            nc.tensor.transpose(_r(at_ps[:, 384:512]), _r(att[:, 384:512]), _r(ident[:]))

            xt_ps = ps.tile([C, W], F32, tag="qk")
            nc.tensor.transpose(_r(xt_ps[:, 0:128]), _r(xt[:, 0:128]), _r(ident[:]))
            nc.tensor.transpose(_r(xt_ps[:, 128:256]), _r(xt[:, 128:256]), _r(ident[:]))

            aT = sbT.tile([C, 2 * W], F32R, tag="aT")
            nc.scalar.copy(_r(aT[:]), at_ps[:])
            xT = sbT.tile([C, W], F32R, tag="xT")
            nc.vector.tensor_copy(_r(xT[:]), xt_ps[:])

            if r % RS == 0:
                o_ps = ps.tile([C, RS * W], F32, tag="out")
            os_ = o_ps[:, (r % RS) * W : (r % RS + 1) * W]
            nc.tensor.matmul(os_, _r(xT[:, 0:128]), _r(aT[:, 0:W]), start=True, stop=False)
            nc.tensor.matmul(
                os_, _r(xT[:, 128:256]), _r(aT[:, W : 2 * W]), start=False, stop=True
            )
            if r % RS == RS - 1:
                o_sb = sbT.tile([C, RS * W], F32, tag="osb")
                nc.any.tensor_copy(o_sb[:], o_ps[:])
                nc.sync.dma_start(out_ext[:, r - RS + 1 : r + 1, :], o_sb[:])

    nc.compile()
    return nc


def host_consts(Wq, bq, Wk, bk):
    sc = 1.0 / np.sqrt(np.float32(C))
    wqt = np.ascontiguousarray(Wq.astype(np.float32).T * sc)
    bqv = np.ascontiguousarray((bq.astype(np.float32) * sc).reshape(C, 1))
    wkt = np.ascontiguousarray(Wk.astype(np.float32).T)
    bkv = np.ascontiguousarray(bk.astype(np.float32).reshape(C, 1))

    ident = np.eye(C, dtype=np.float32)
    bandmask = np.full((C, 2 * W), NEG, dtype=np.float32)
    oob = np.zeros((C, 2), dtype=np.float32)
    for t in range(2):
        for p in range(C):
            w = t * 128 + p
            lo, hi = max(0, w - R), min(W, w + R + 1)
            bandmask[p, t * W + lo : t * W + hi] = 0.0
            oob[p, t] = max(0, R - w) + max(0, w - (W - 1 - R))
    return wqt, bqv, wkt, bkv, bandmask, oob, ident


def core_inputs(feature, position, Wq, bq, Wk, bk):
    wqt, bqv, wkt, bkv, bandmask, oob, ident = host_consts(Wq, bq, Wk, bk)
    in_maps = []
    for i in range(NCORES):
        b = i // CORES_PER_B
        h0 = (i % CORES_PER_B) * ROWS
        in_maps.append(
            {
                "feature": np.ascontiguousarray(
                    feature[b, :, h0 : h0 + ROWS, :], dtype=np.float32
                ),
                "position": np.ascontiguousarray(
                    position[b, :, h0 : h0 + ROWS, :], dtype=np.float32
                ),
                "wqt": wqt,
                "ident": ident,
                "wkt": wkt,
                "bqv": bqv,
                "bkv": bkv,
                "bandmask": bandmask,
                "oob": oob,
            }
        )
    return in_maps


def kernel(feature, position, Wq, bq, Wk, bk):
    in_maps = core_inputs(feature, position, Wq, bq, Wk, bk)
    nc = build_nc()
    res = run_bass_kernel_spmd(nc, in_maps, list(range(NCORES)))
    out = np.empty((B, C, H, W), dtype=np.float32)
    for i in range(NCORES):
        b = i // CORES_PER_B
        h0 = (i % CORES_PER_B) * ROWS
        out[b, :, h0 : h0 + ROWS, :] = res.results[i]["out"]
    return out
